# revision 6
# baseline (speedup 1.0000x reference)
"""Bass/Trainium2 kernel for nn_KernelEdges (gnn_message_passing).

Computes A = exp((g_i + g_j - 2*Xf@Xf.T)/sigma^2) with zeroed diagonal,
broadcast to all B batch slots, where Xf = X.transpose(1,0,2).reshape(N, B*d).

Sharding: rows of the NxN pairwise matrix are split across 8 NeuronCores
(256 rows each).  Each core receives the full transposed operand
XT = Xf.T [B*d, N] in bf16, column-rotated so the core's own row-block
sits at columns 0:256 (one shared program; the stationary matmul operand
is a plain slice of the xt tile).  Each core writes its [N/8, N] tile of
the pairwise matrix ONCE in bf16; the host un-rotates, upcasts, zeroes
the diagonal and broadcasts to the B identical batch slots at gather
time (the batch dim of the reference output is an exact broadcast).

Pipelining: the input is streamed in [128, 512] column pieces, ordered
nb-major across the four k-tiles, so the PSUM chain for column block nb
can stop as soon as its column slice of all k-tiles has arrived.  ACT
(exp) and the output-store DMAs then overlap the remaining input stream.
A burst of dummy warm-up matmuls at program start keeps the PE busy so
the HAM clock gate lifts the PE from 1.2 to 2.4 GHz before the real
matmuls run.  Input pieces alternate between the two HWDGE rings
(scalar + sync); output chunks go out on the sync ring as they're ready.

Per-core device work per column block nb:
  psum[mt,nb] = (-1/2*ones).T @ g_row[nb]                      (rank-1: -g_j/2)
              + sum_q xt_q[:, mt-slice].T @ xt_q[:, nb]        (Gram)
  A[:, nb] = exp(-2/sigma^2 * psum + g_i/sigma^2)              (ACT, bias/row)
"""

import numpy as np

B, N, D = 8, 2048, 64
NCORES = 8
R = N // NCORES          # 256 rows per core
KD = B * D               # 512 contraction dim
PC = 512                 # column-piece width (one PSUM bank of fp32)
NP = N // PC             # 4 column blocks
NMT = R // 128           # 2 m-tiles per core
NQ = KD // 128           # 4 k-tiles
GK = 2                   # g carried as hi+lo rows in bf16
NWARM = 6                # PE warm-up matmuls (~2.6us at cold clock)


def _build_program(inv_s2):
    import concourse.bass as bass
    import concourse.tile as tile
    from concourse import bacc, mybir

    f32 = mybir.dt.float32
    bf16 = mybir.dt.bfloat16

    nc = bacc.Bacc(
        "TRN2", target_bir_lowering=False, debug=False, num_devices=NCORES
    )

    xt_d = nc.dram_tensor("xt", [KD, N], bf16, kind="ExternalInput").ap()
    grow_d = nc.dram_tensor("grow", [GK, N], bf16, kind="ExternalInput").ap()
    bias_d = nc.dram_tensor("bias", [128, NMT], f32, kind="ExternalInput").ap()
    out_d = nc.dram_tensor("out", [R, N], bf16, kind="ExternalOutput").ap()

    with tile.TileContext(nc) as tc:
        with (
            tc.tile_pool(name="persist", bufs=1) as persist,
            tc.tile_pool(name="apool", bufs=1) as apool,
            tc.tile_pool(name="psum", bufs=1, space="PSUM") as pspool,
        ):
            grow_sb = persist.tile([GK, N], bf16, name="grow")
            bias_sb = persist.tile([128, NMT], f32, name="bias")
            neg_half = persist.tile([GK, PC], bf16, name="neg_half")
            # -0.5 bf16; also the warm-up operand.  DVE memset: the vector
            # engine clears the preamble barrier early and is otherwise idle.
            nc.vector.memset(neg_half[:].bitcast(mybir.dt.uint16), 0xBF00)

            xt_sb = [
                persist.tile([128, N], bf16, name=f"xt{q}") for q in range(NQ)
            ]

            # ---- input DMAs ----
            # pieces in nb-major arrival order, alternating between the two
            # HWDGE rings; grow/bias (tiny) lead their rings.
            nc.sync.dma_start(grow_sb[:], grow_d[:])
            nc.scalar.dma_start(bias_sb[:], bias_d[:])
            for i in range(NP * NQ):
                nb, q = divmod(i, NQ)
                sl = slice(nb * PC, (nb + 1) * PC)
                eng = nc.scalar if i % 2 == 0 else nc.sync
                eng.dma_start(xt_sb[q][:, sl], xt_d[q * 128:(q + 1) * 128, sl])

            # ---- PSUM chains ----
            ps = {
                (mt, nb): pspool.tile([128, PC], f32, name=f"ps{mt}{nb}")
                for mt in range(NMT)
                for nb in range(NP)
            }
            # PE warm-up: dummy matmuls (discarded) to lift the HAM clock
            # gate before the real work; depend only on the memset.
            for w in range(NWARM):
                mt, nb = divmod(w % (NMT * NP), NP)
                nc.tensor.matmul(
                    ps[mt, nb][:],
                    neg_half[:, 0:128],
                    neg_half[:],
                    start=True,
                    stop=True,
                )

            a_sb = {
                mt: apool.tile([128, N], bf16, name=f"a{mt}")
                for mt in range(NMT)
            }
            for nb in range(NP):
                sl = slice(nb * PC, (nb + 1) * PC)
                for q in range(NQ):
                    for mt in range(NMT):
                        # rotated layout: this core's own rows are cols 0:R.
                        # q0 starts the chain (no grow dependency); the
                        # cheap K=2 rank-1 (-g_j/2) is slotted after q0.
                        nc.tensor.matmul(
                            ps[mt, nb][:],
                            xt_sb[q][:, mt * 128:(mt + 1) * 128],
                            xt_sb[q][:, sl],
                            start=q == 0,
                            stop=q == NQ - 1,
                        )
                    if q == 0:
                        for mt in range(NMT):
                            nc.tensor.matmul(
                                ps[mt, nb][:],
                                neg_half[:, 0:128],
                                grow_sb[:, sl],
                                start=False,
                                stop=False,
                            )
                # the final column block's ACT/store is split in half so the
                # tail after the last matmul is as short as possible
                nsp = 2 if nb == NP - 1 else 1
                for sp in range(nsp):
                    w = PC // nsp
                    for mt in range(NMT):
                        ssl = slice(nb * PC + sp * w, nb * PC + (sp + 1) * w)
                        psl = slice(sp * w, (sp + 1) * w)
                        nc.scalar.activation(
                            a_sb[mt][:, ssl],
                            ps[mt, nb][:, psl],
                            mybir.ActivationFunctionType.Exp,
                            bias=bias_sb[:, mt:mt + 1],
                            scale=-2.0 * inv_s2,
                        )
                    for mt in range(NMT):
                        ssl = slice(nb * PC + sp * w, nb * PC + (sp + 1) * w)
                        eng = nc.sync if (nb * 2 + mt) % 2 == 0 else nc.scalar
                        eng.dma_start(
                            out_d[mt * 128:(mt + 1) * 128, ssl],
                            a_sb[mt][:, ssl],
                        )

    nc.compile()
    return nc


def _prepare(X, log_sigma):
    """Host prep: returns (inv_s2, in_maps) for run_bass_kernel_spmd."""
    import ml_dtypes

    X = np.ascontiguousarray(X, dtype=np.float32)
    assert X.shape == (B, N, D), X.shape

    sigma = float(np.exp(np.float32(log_sigma)))
    inv_s2 = 1.0 / (sigma * sigma)

    # XT[b*D+f, n] = X[b, n, f]
    XT = np.ascontiguousarray(X.transpose(0, 2, 1).reshape(KD, N))
    g = np.einsum("kn,kn->n", XT, XT).astype(np.float32)  # [N]

    XTb = XT.astype(ml_dtypes.bfloat16)
    g_hi = g.astype(ml_dtypes.bfloat16)
    g_lo = (g - g_hi.astype(np.float32)).astype(ml_dtypes.bfloat16)
    grow_np = np.stack([g_hi, g_lo])  # [2, N]

    in_maps = []
    for c in range(NCORES):
        r0 = c * R
        bias_np = np.empty((128, NMT), dtype=np.float32)
        for mt in range(NMT):
            bias_np[:, mt] = g[r0 + mt * 128: r0 + (mt + 1) * 128] * inv_s2
        in_maps.append({
            # rotate columns so this core's own rows land at cols 0:R
            "xt": np.ascontiguousarray(np.roll(XTb, -r0, axis=1)),
            "grow": np.ascontiguousarray(np.roll(grow_np, -r0, axis=1)),
            "bias": bias_np,
        })
    return inv_s2, in_maps


def kernel(X, log_sigma):
    from concourse.bass_utils import run_bass_kernel_spmd

    inv_s2, in_maps = _prepare(X, log_sigma)
    nc = _build_program(inv_s2)
    res = run_bass_kernel_spmd(nc, in_maps, list(range(NCORES)))
    rows = []
    for c in range(NCORES):
        t = res.results[c]["out"].astype(np.float32)  # [R, N], rotated cols
        rows.append(np.roll(t, c * R, axis=1))
    A = np.concatenate(rows, axis=0)  # [N, N]
    idx = np.arange(N)
    A[idx, idx] = 0.0
    return np.ascontiguousarray(np.broadcast_to(A[None, :, :], (B, N, N)))


# revision 7
# speedup vs baseline: 1.0225x; 1.0225x over previous
"""Bass/Trainium2 kernel for nn_KernelEdges (gnn_message_passing).

Computes A = exp((g_i + g_j - 2*Xf@Xf.T)/sigma^2) with zeroed diagonal,
broadcast to all B batch slots, where Xf = X.transpose(1,0,2).reshape(N, B*d).

Sharding: rows of the NxN pairwise matrix are split across 8 NeuronCores
(256 rows each).  Each core receives the full transposed operand
XT = Xf.T [B*d, N] in bf16, column-rotated so the core's own row-block
sits at columns 0:256 (one shared program; the stationary matmul operand
is a plain slice of the input tile).  Each core writes its [N/8, N] tile
ONCE in bf16; the host un-rotates, upcasts, applies the per-column
exp(g_j/sigma^2) factor (A factorizes as exp((g_i-2xx)/s2)*exp(g_j/s2)),
zeroes the diagonal and broadcasts to the B identical batch slots at
gather time (the batch dim of the reference output is an exact
broadcast).

Device work is a pure Gram matrix + exp:
  psum[mt, blk] = sum_q xt_q[:, mt-slice].T @ xt_q[:, blk]
  A[:, blk]     = exp(-2/sigma^2 * psum + g_i/sigma^2)    (ACT, bias/row)

Performance structure:
 - ONE packed DRAM input per core, [128, 4 + 4*N] bf16: 4 bias columns
   (the f32 g_i/sigma^2 pair, bit-viewed) then the 4 k-tiles packed
   q-major per column block, so each column block is a single
   contiguous-per-partition DMA (dma_start instructions cost the issuing
   engine ~0.7us each, so few big DMAs beat many small ones).
 - Column blocks of width [512,512,512,256,256] stream in on the scalar
   ring; each block's PSUM chains stop as soon as the block lands, so
   the exp ACTs and the output stores overlap the input stream.  The
   final blocks are narrow to shorten the post-matmul tail.
 - Dummy warm-up matmuls bridge the preamble-to-first-block window so
   the HAM clock gate holds the PE at 2.4 GHz for the real matmuls.
 - Outputs leave as 4 [128, 1024] bf16 DMAs on the sync ring.
"""

import numpy as np

B, N, D = 8, 2048, 64
NCORES = 8
R = N // NCORES          # 256 rows per core
KD = B * D               # 512 contraction dim
NMT = R // 128           # 2 m-tiles per core
NQ = KD // 128           # 4 k-tiles
BW = [512, 512, 512, 256, 256]          # column-block widths
BC = [0, 512, 1024, 1536, 1792]         # column-block starts
NBLK = len(BW)
OFF = 4                  # bias columns at the head of the packed input
NWARM = 7                # PE warm-up matmuls (~3us at cold clock)


def _build_program(inv_s2):
    import concourse.bass as bass
    import concourse.tile as tile
    from concourse import bacc, mybir

    f32 = mybir.dt.float32
    bf16 = mybir.dt.bfloat16

    nc = bacc.Bacc(
        "TRN2", target_bir_lowering=False, debug=False, num_devices=NCORES
    )

    xt_d = nc.dram_tensor(
        "xt2", [128, OFF + NQ * N], bf16, kind="ExternalInput"
    ).ap()
    out_d = nc.dram_tensor("out", [R, N], bf16, kind="ExternalOutput").ap()

    # packed column start of each block
    BS = [OFF + NQ * c for c in BC]

    with tile.TileContext(nc) as tc:
        with (
            tc.tile_pool(name="persist", bufs=1) as persist,
            tc.tile_pool(name="apool", bufs=1) as apool,
            tc.tile_pool(name="psum", bufs=1, space="PSUM") as pspool,
        ):
            neg_half = persist.tile([2, 512], bf16, name="warmops")
            nc.vector.memset(neg_half[:].bitcast(mybir.dt.uint16), 0xBF00)

            xt_all = persist.tile([128, OFF + NQ * N], bf16, name="xt")
            # block DMAs, all on the scalar ring (sync ring is reserved for
            # the output stores so they never queue behind input blocks);
            # block 0 carries the bias columns along.
            for b in range(NBLK):
                lo = BS[b] - (OFF if b == 0 else 0)
                hi = BS[b] + NQ * BW[b]
                nc.scalar.dma_start(xt_all[:, lo:hi], xt_d[:, lo:hi])

            bias_ap = xt_all[:, 0:OFF].bitcast(f32)

            ps = {
                mt: pspool.tile([128, N], f32, name=f"ps{mt}")
                for mt in range(NMT)
            }
            # PE warm-up: dummy matmuls (results discarded) so the HAM
            # clock gate lifts the PE to 2.4 GHz before the real work.
            for w in range(NWARM):
                nc.tensor.matmul(
                    ps[w % NMT][:, (w % NQ) * 512:(w % NQ + 1) * 512],
                    neg_half[:, 0:128],
                    neg_half[:],
                    start=True,
                    stop=True,
                )

            a_sb = {
                mt: apool.tile([128, N], bf16, name=f"a{mt}")
                for mt in range(NMT)
            }
            for b in range(NBLK):
                c, w, s = BC[b], BW[b], BS[b]
                for q in range(NQ):
                    for mt in range(NMT):
                        # rotated layout: this core's own rows are the
                        # first 256 data columns of the q-run in block 0
                        nc.tensor.matmul(
                            ps[mt][:, c:c + w],
                            xt_all[:, OFF + q * 512 + mt * 128:
                                   OFF + q * 512 + (mt + 1) * 128],
                            xt_all[:, s + q * w:s + (q + 1) * w],
                            start=q == 0,
                            stop=q == NQ - 1,
                        )
                for mt in range(NMT):
                    nc.scalar.activation(
                        a_sb[mt][:, c:c + w],
                        ps[mt][:, c:c + w],
                        mybir.ActivationFunctionType.Exp,
                        bias=bias_ap[:, mt:mt + 1],
                        scale=-2.0 * inv_s2,
                    )
                if b in (1, NBLK - 1):
                    h0 = 0 if b == 1 else 1024
                    for mt in range(NMT):
                        nc.sync.dma_start(
                            out_d[mt * 128:(mt + 1) * 128, h0:h0 + 1024],
                            a_sb[mt][:, h0:h0 + 1024],
                        )

    nc.compile()
    return nc


def _prepare(X, log_sigma):
    """Host prep: returns (inv_s2, g, in_maps) for run_bass_kernel_spmd."""
    import ml_dtypes

    X = np.ascontiguousarray(X, dtype=np.float32)
    assert X.shape == (B, N, D), X.shape

    sigma = float(np.exp(np.float32(log_sigma)))
    inv_s2 = 1.0 / (sigma * sigma)

    # XT[b*D+f, n] = X[b, n, f]
    XT = np.ascontiguousarray(X.transpose(0, 2, 1).reshape(KD, N))
    g = np.einsum("kn,kn->n", XT, XT).astype(np.float32)  # [N]
    XTb = XT.astype(ml_dtypes.bfloat16)

    in_maps = []
    for cix in range(NCORES):
        r0 = cix * R
        # rotate columns so this core's own rows land at cols 0:R
        XTr = np.roll(XTb, -r0, axis=1)
        packed = np.empty((128, OFF + NQ * N), dtype=ml_dtypes.bfloat16)
        bias_np = np.empty((128, NMT), dtype=np.float32)
        for mt in range(NMT):
            bias_np[:, mt] = g[r0 + mt * 128: r0 + (mt + 1) * 128] * inv_s2
        packed[:, 0:OFF] = bias_np.view(ml_dtypes.bfloat16)
        for b in range(NBLK):
            c, w = BC[b], BW[b]
            s = OFF + NQ * c
            sub = XTr[:, c:c + w].reshape(NQ, 128, w).transpose(1, 0, 2)
            packed[:, s:s + NQ * w] = sub.reshape(128, NQ * w)
        in_maps.append({"xt2": packed})
    return inv_s2, g, in_maps


def kernel(X, log_sigma):
    from concourse.bass_utils import run_bass_kernel_spmd

    inv_s2, g, in_maps = _prepare(X, log_sigma)
    nc = _build_program(inv_s2)
    res = run_bass_kernel_spmd(nc, in_maps, list(range(NCORES)))
    rows = []
    for c in range(NCORES):
        t = res.results[c]["out"].astype(np.float32)  # [R, N], rotated cols
        rows.append(np.roll(t, c * R, axis=1))
    A = np.concatenate(rows, axis=0)  # [N, N] = exp((g_i - 2*x_i.x_j)/s2)
    A *= np.exp(g * inv_s2)[None, :]  # per-column exp(g_j/s2) factor
    idx = np.arange(N)
    A[idx, idx] = 0.0
    return np.ascontiguousarray(np.broadcast_to(A[None, :, :], (B, N, N)))
